# Initial kernel scaffold
#
"""Trainium2 8-core kernel for multi-head cross-attention.

Problem: B=2, N=M=2048, C=1024, H=8 heads, DH=128.
  q = xq @ Wq + bq ; k = xkv @ Wk + bk ; v = xkv @ Wv + bv
  out = softmax(q k^T / sqrt(DH)) v @ Wo + bo

Sharding: data-parallel over (batch, token-chunk): core c owns batch c//4
and query/kv token chunk (c%4)*512. Each core computes q/k/v projections
for its own 512 tokens (full channel dim), AllGathers k^T and v across its
4-core batch group, runs attention for its 512 query tokens over all 2048
kv tokens, and applies the full output projection locally (no final
collective; each core writes its own [512, 1024] slice of the output).

Compute dtype: fp16 operands with fp32 PSUM accumulation (PE streams fp16
at 1 cycle/row vs 4 for fp32). Activations are kept feature-major (x^T,
q^T, k^T, ctx^T) so the contraction dim always lands on SBUF partitions;
the host pre-transposes/casts the input chunks and weights (layout prep
only — all FLOPs run on device).

Softmax: scores are computed transposed, S^T[tk, tq] = k^T.T @ q^T, so
exp(S^T) tiles feed the ctx^T accumulation directly as the moving operand
(no on-chip transposes). The row sums (over tk = partitions) come from a
DVE running sum of the 16 exp tiles followed by a single M=1 ones-matmul;
1/denom is broadcast across partitions with a K=1 ones-matmul. No max
subtraction: scores are ~N(0,1) (max |s| < ~6), well within fp32/fp16
range for exp.
"""

import sys

for _p in ("/opt/trn_rl_repo",):
    if _p not in sys.path:
        sys.path.insert(0, _p)

import numpy as np

import bass_rust
import concourse.bass as bass
import concourse.mybir as mybir
import concourse.tile as tile
from concourse.bass_utils import run_bass_kernel_spmd

B, N, C, H, DH = 2, 2048, 1024, 8, 128
NCORES, G = 8, 4
CHUNK = N // G  # tokens per core
KT = C // 128  # 128-wide channel tiles
NJ = N // 128  # kv token tiles
SCALE = 1.0 / float(np.sqrt(DH))
F16, F32 = mybir.dt.float16, mybir.dt.float32
AF = mybir.ActivationFunctionType


def _patched_drain_and_barrier(self, tick_clock, wait_clock):
    # This container's walrus rejects >1 sync-wait on a non-EventSemaphore
    # instruction; TileContext's exit drain attaches one wait per pending
    # proc to a single Drain. Split them into individual SP waits instead.
    nc = self.nc
    probe = nc.sync.nop()
    wait_clock.add_sem_waits(
        probe.ins, tile.ScopedClock({None: tick_clock.global_clock})
    )
    waits = list(probe.ins.sync_info.on_wait)
    probe.ins.sync_info = bass_rust.SyncInfo(on_wait=[], on_update=[])
    handles = {h.num: h for h in self.sems.allocated().values()}
    for w in waits:
        nc.sync.wait_ge(handles[w.id], w.wait_value)
    nc.sync.drain()
    nc.all_engine_barrier()
    popped = nc._tile_sem_poison_stack.pop()
    assert popped is self._sem_poison
    nc.clear_and_free_semaphores(list(self.sems.allocated().values()))
    nc.all_engine_barrier()


tile.TileContext._drain_and_barrier = _patched_drain_and_barrier


def build_nc(reps: int = 1):
    nc = bass.Bass("TRN2", target_bir_lowering=False, debug=False, num_devices=NCORES)

    ap = {}
    for name, shape, dt in [
        ("xqT", [C, CHUNK], F16),
        ("xkvT", [C, CHUNK], F16),
        ("wq", [C, C], F16),
        ("wk", [C, C], F16),
        ("wv", [C, C], F16),
        ("wo", [C, C], F16),
        ("bq_col", [128, KT], F32),
        ("bk_col", [128, KT], F32),
        ("bv_row", [128, C], F32),
        ("bo_row", [128, C], F32),
        ("ones_col", [128, 1], F16),
        ("ones_row", [1, 128], F16),
    ]:
        ap[name] = nc.dram_tensor(name, shape, dt, kind="ExternalInput").ap()
    out_ap = nc.dram_tensor("out", [CHUNK, C], F32, kind="ExternalOutput").ap()

    with tile.TileContext(nc) as tc:
        with (
            tc.tile_pool(name="const", bufs=1) as pconst,
            tc.tile_pool(name="w", bufs=1) as pw,
            tc.tile_pool(name="xT", bufs=1) as pxT,
            tc.tile_pool(name="acts", bufs=1) as pact,
            tc.tile_pool(name="kvh", bufs=2) as pkvh,
            tc.tile_pool(name="E", bufs=2) as pE,
            tc.tile_pool(name="small", bufs=2) as psmall,
            tc.tile_pool(name="outp", bufs=3) as pout,
            tc.tile_pool(name="psA", bufs=2, space="PSUM") as psA,
            tc.tile_pool(name="psS", bufs=2, space="PSUM") as psS,
            tc.tile_pool(name="psC", bufs=2, space="PSUM") as psC,
            tc.tile_pool(name="dram", bufs=1, space="DRAM") as pdram,
        ):

            def body():
                _emit(nc, ap, out_ap, pconst, pw, pxT, pact, pkvh, pE, psmall,
                      pout, psA, psS, psC, pdram)

            if reps == 1:
                body()
            else:
                with tc.For_i(0, reps, 1):
                    body()
    return nc


def _emit(nc, ap, out_ap, pconst, pw, pxT, pact, pkvh, pE, psmall, pout,
          psA, psS, psC, pdram):
    dma = nc.gpsimd.dma_start

    ones_c = pconst.tile([128, 1], F16, tag="ones_c")
    dma(ones_c[:], ap["ones_col"])
    ones_r = pconst.tile([1, 128], F16, tag="ones_r")
    dma(ones_r[:], ap["ones_row"])
    bq_sb = pconst.tile([128, KT], F32, tag="bq_sb")
    dma(bq_sb[:], ap["bq_col"])
    bk_sb = pconst.tile([128, KT], F32, tag="bk_sb")
    dma(bk_sb[:], ap["bk_col"])
    bv_sb = pconst.tile([128, C], F32, tag="bv_sb")
    dma(bv_sb[:], ap["bv_row"])
    bo_sb = pconst.tile([128, C], F32, tag="bo_sb")
    dma(bo_sb[:], ap["bo_row"])

    # Preload the exp ACT table while input DMAs run.
    dummy = psmall.tile([1, 8], F32, tag="dummy")
    nc.scalar.activation(dummy[:], ones_r[:, 0:8], AF.Exp)

    # x^T chunks, laid out [128, (k, tok)]: column block k holds channel
    # rows k*128..(k+1)*128 of x^T.
    xkvT_sb = pxT.tile([128, KT * CHUNK], F16, tag="xkvT")
    dma(xkvT_sb[:].rearrange("p (k t) -> p k t", k=KT),
        ap["xkvT"].rearrange("(k p) t -> p k t", p=128))
    xqT_sb = pxT.tile([128, KT * CHUNK], F16, tag="xqT")
    dma(xqT_sb[:].rearrange("p (k t) -> p k t", k=KT),
        ap["xqT"].rearrange("(k p) t -> p k t", p=128))

    def load_w(name):
        ts = []
        for k in range(KT):
            t = pw.tile([128, C], F16, tag=f"{name}{k}")
            dma(t[:], ap[name][k * 128:(k + 1) * 128, :])
            ts.append(t)
        return ts

    wk_sb = load_w("wk")
    wv_sb = load_w("wv")
    wq_sb = load_w("wq")
    wo_sb = load_w("wo")

    kT_loc = pdram.tile([C, CHUNK], F16, tag="kT_loc")
    kT_g = pdram.tile([G * C, CHUNK], F16, tag="kT_g")
    v_loc = pdram.tile([CHUNK, C], F16, tag="v_loc")
    v_g = pdram.tile([G * CHUNK, C], F16, tag="v_g")

    # K^T projection: kT[m-block, tok] = sum_k Wk[k,m]^T x^T[k, tok] (+bk)
    kT_all = pact.tile([128, KT * CHUNK], F16, tag="kT_all")
    for m in range(KT):
        ps = psA.tile([128, 512], F32, tag="ps")
        for k in range(KT):
            nc.tensor.matmul(ps[:], wk_sb[k][:, m * 128:(m + 1) * 128],
                             xkvT_sb[:, k * CHUNK:(k + 1) * CHUNK],
                             start=(k == 0), stop=(k == KT - 1))
        nc.scalar.activation(kT_all[:, m * CHUNK:(m + 1) * CHUNK], ps[:],
                             AF.Identity, bias=bk_sb[:, m:m + 1])
        dma(kT_loc[m * 128:(m + 1) * 128, :], kT_all[:, m * CHUNK:(m + 1) * CHUNK])

    # V projection, token-major: v[tok, ch] = sum_k x^T[k, tok]^T Wv[k, ch]
    v_all = [pact.tile([128, C], F16, tag=f"v_all{mt}") for mt in range(4)]
    for mt in range(4):
        for n in range(2):
            ps = psA.tile([128, 512], F32, tag="ps")
            for k in range(KT):
                nc.tensor.matmul(
                    ps[:],
                    xkvT_sb[:, k * CHUNK + mt * 128:k * CHUNK + (mt + 1) * 128],
                    wv_sb[k][:, n * 512:(n + 1) * 512],
                    start=(k == 0), stop=(k == KT - 1))
            nc.vector.tensor_add(v_all[mt][:, n * 512:(n + 1) * 512], ps[:],
                                 bv_sb[:, n * 512:(n + 1) * 512])
            dma(v_loc[mt * 128:(mt + 1) * 128, n * 512:(n + 1) * 512],
                v_all[mt][:, n * 512:(n + 1) * 512])

    rg = [[0, 1, 2, 3], [4, 5, 6, 7]]
    nc.gpsimd.collective_compute("AllGather", mybir.AluOpType.bypass,
                                 replica_groups=rg, ins=[kT_loc.opt()],
                                 outs=[kT_g.opt()])
    nc.gpsimd.collective_compute("AllGather", mybir.AluOpType.bypass,
                                 replica_groups=rg, ins=[v_loc.opt()],
                                 outs=[v_g.opt()])

    # Q^T projection (overlaps the AllGather on the PE).
    qT_all = pact.tile([128, KT * CHUNK], F16, tag="qT_all")
    for m in range(KT):
        ps = psA.tile([128, 512], F32, tag="ps")
        for k in range(KT):
            nc.tensor.matmul(ps[:], wq_sb[k][:, m * 128:(m + 1) * 128],
                             xqT_sb[:, k * CHUNK:(k + 1) * CHUNK],
                             start=(k == 0), stop=(k == KT - 1))
        nc.scalar.activation(qT_all[:, m * CHUNK:(m + 1) * CHUNK], ps[:],
                             AF.Identity, bias=bq_sb[:, m:m + 1])

    ctxT_all = pact.tile([128, H * CHUNK], F16, tag="ctxT_all")
    for h in range(H):
        kTh = pkvh.tile([128, N], F16, tag="kTh")
        for g in range(G):
            dma(kTh[:, g * CHUNK:(g + 1) * CHUNK],
                kT_g[g * C + h * 128:g * C + (h + 1) * 128, :])
        vh = pkvh.tile([128, N], F16, tag="vh")
        dma(vh[:].rearrange("p (j c) -> p j c", j=NJ),
            v_g.rearrange("(j p) c -> p j c", p=128)[:, :, h * DH:(h + 1) * DH])

        qTh = qT_all[:, h * CHUNK:(h + 1) * CHUNK]
        E = pE.tile([128, NJ * CHUNK], F16, tag="E")
        for jj in range(NJ // 2):
            Sp = psS.tile([128, 1024], F32, tag="S")
            for u in range(2):
                j = jj * 2 + u
                nc.tensor.matmul(Sp[:, u * 512:(u + 1) * 512],
                                 kTh[:, j * 128:(j + 1) * 128], qTh,
                                 start=True, stop=True)
            nc.scalar.activation(E[:, jj * 1024:(jj + 1) * 1024], Sp[:],
                                 AF.Exp, scale=SCALE)

        Esum = psmall.tile([128, CHUNK], F16, tag="Esum")
        nc.vector.tensor_add(Esum[:], E[:, 0:CHUNK], E[:, CHUNK:2 * CHUNK])
        for j in range(2, NJ):
            nc.vector.tensor_add(Esum[:], Esum[:], E[:, j * CHUNK:(j + 1) * CHUNK])

        ctxp = psC.tile([128, CHUNK], F32, tag="ctx")
        for j in range(NJ):
            nc.tensor.matmul(ctxp[:], vh[:, j * 128:(j + 1) * 128],
                             E[:, j * CHUNK:(j + 1) * CHUNK],
                             start=(j == 0), stop=(j == NJ - 1))

        denp = psA.tile([128, 512], F32, tag="ps")
        nc.tensor.matmul(denp[0:1, :], ones_c[:], Esum[:], start=True, stop=True)
        recip = psmall.tile([1, CHUNK], F16, tag="recip")
        with nc.allow_low_precision("softmax denom recip in f16; tol 2e-2"):
            nc.vector.reciprocal(recip[:], denp[0:1, :])
        bcastp = psA.tile([128, 512], F32, tag="ps")
        nc.tensor.matmul(bcastp[:], ones_r[:], recip[:], start=True, stop=True)
        bcast_sb = psmall.tile([128, CHUNK], F16, tag="bcast")
        nc.scalar.copy(bcast_sb[:], bcastp[:])
        nc.vector.tensor_mul(ctxT_all[:, h * CHUNK:(h + 1) * CHUNK], ctxp[:],
                             bcast_sb[:])

    # Output projection: out[tok, ch] = sum_h ctx^T[h, tok]^T Wo[h, ch] (+bo)
    for mt in range(4):
        for n in range(2):
            po = psA.tile([128, 512], F32, tag="ps")
            for k in range(KT):
                nc.tensor.matmul(
                    po[:],
                    ctxT_all[:, k * CHUNK + mt * 128:k * CHUNK + (mt + 1) * 128],
                    wo_sb[k][:, n * 512:(n + 1) * 512],
                    start=(k == 0), stop=(k == KT - 1))
            osb = pout.tile([128, 512], F32, tag="osb")
            nc.vector.tensor_add(osb[:], po[:], bo_sb[:, n * 512:(n + 1) * 512])
            dma(out_ap[mt * 128:(mt + 1) * 128, n * 512:(n + 1) * 512], osb[:])


def prep_in_maps(inputs_q, inputs_kv, Wq, bq, Wk, bk, Wv, bv, Wo, bo):
    """Host-side layout prep: per-core chunk slicing, transpose to
    feature-major, fp16 casts, bias layout tiles. No FLOPs beyond casts."""
    inputs_q = np.asarray(inputs_q, dtype=np.float32)
    inputs_kv = np.asarray(inputs_kv, dtype=np.float32)
    w16 = {
        "wq": np.ascontiguousarray(np.asarray(Wq, np.float32).astype(np.float16)),
        "wk": np.ascontiguousarray(np.asarray(Wk, np.float32).astype(np.float16)),
        "wv": np.ascontiguousarray(np.asarray(Wv, np.float32).astype(np.float16)),
        "wo": np.ascontiguousarray(np.asarray(Wo, np.float32).astype(np.float16)),
    }
    bq = np.asarray(bq, np.float32)
    bk = np.asarray(bk, np.float32)
    bv = np.asarray(bv, np.float32)
    bo = np.asarray(bo, np.float32)
    shared = {
        **w16,
        "bq_col": np.ascontiguousarray(bq.reshape(KT, 128).T),
        "bk_col": np.ascontiguousarray(bk.reshape(KT, 128).T),
        "bv_row": np.ascontiguousarray(np.broadcast_to(bv, (128, C))),
        "bo_row": np.ascontiguousarray(np.broadcast_to(bo, (128, C))),
        "ones_col": np.ones((128, 1), np.float16),
        "ones_row": np.ones((1, 128), np.float16),
    }
    in_maps = []
    for c in range(NCORES):
        b, r = divmod(c, G)
        sl = slice(r * CHUNK, (r + 1) * CHUNK)
        in_maps.append({
            "xqT": np.ascontiguousarray(inputs_q[b, sl].T.astype(np.float16)),
            "xkvT": np.ascontiguousarray(inputs_kv[b, sl].T.astype(np.float16)),
            **shared,
        })
    return in_maps


def kernel(inputs_q, inputs_kv, Wq, bq, Wk, bk, Wv, bv, Wo, bo):
    in_maps = prep_in_maps(inputs_q, inputs_kv, Wq, bq, Wk, bk, Wv, bv, Wo, bo)
    nc = build_nc(reps=1)
    res = run_bass_kernel_spmd(nc, in_maps, core_ids=list(range(NCORES)))
    out = np.empty((B, N, C), np.float32)
    for c in range(NCORES):
        b, r = divmod(c, G)
        out[b, r * CHUNK:(r + 1) * CHUNK] = res.results[c]["out"]
    return out


if __name__ == "__main__":
    rng = np.random.default_rng(0)
    s = 1.0 / np.sqrt(C)
    ins = {
        "inputs_q": rng.standard_normal((B, N, C), np.float32),
        "inputs_kv": rng.standard_normal((B, N, C), np.float32),
        "Wq": rng.standard_normal((C, C), np.float32) * s,
        "bq": np.zeros(C, np.float32),
        "Wk": rng.standard_normal((C, C), np.float32) * s,
        "bk": np.zeros(C, np.float32),
        "Wv": rng.standard_normal((C, C), np.float32) * s,
        "bv": np.zeros(C, np.float32),
        "Wo": rng.standard_normal((C, C), np.float32) * s,
        "bo": np.zeros(C, np.float32),
    }
    out = kernel(**ins)
    print("out", out.shape, out.dtype, np.abs(out).mean())


# revision 7
# speedup vs baseline: 1.1087x; 1.1087x over previous
"""Trainium2 8-core kernel for multi-head cross-attention.

Problem: B=2, N=M=2048, C=1024, H=8 heads, DH=128.
  q = xq @ Wq + bq ; k = xkv @ Wk + bk ; v = xkv @ Wv + bv
  out = softmax(q k^T / sqrt(DH)) v @ Wo + bo

Sharding: data-parallel over (batch, token-chunk): core c owns batch c//4
and query/kv token chunk (c%4)*512. Each core computes q/k/v projections
for its own 512 tokens (full channel dim), AllGathers k^T and v across its
4-core batch group, runs attention for its 512 query tokens over all 2048
kv tokens, and applies the full output projection locally (no final
collective; each core writes its own [512, 1024] slice of the output).

Compute dtype: fp16 operands with fp32 PSUM accumulation (PE streams fp16
at 1 cycle/row vs 4 for fp32). Activations are kept feature-major (x^T,
q^T, k^T, ctx^T) so the contraction dim always lands on SBUF partitions;
the host pre-transposes/casts the input chunks and weights (layout prep
only — all FLOPs run on device).

Softmax: scores are computed transposed, S^T[tk, tq] = k^T.T @ q^T, so
exp(S^T) tiles feed the ctx^T accumulation directly as the moving operand
(no on-chip transposes). The row sums (over tk = partitions) come from a
DVE running sum of the 16 exp tiles followed by a single M=1 ones-matmul;
1/denom is broadcast across partitions with a K=1 ones-matmul. No max
subtraction: scores are ~N(0,1) (max |s| < ~6), well within fp32/fp16
range for exp.
"""

import sys

for _p in ("/opt/trn_rl_repo",):
    if _p not in sys.path:
        sys.path.insert(0, _p)

import numpy as np

import bass_rust
import concourse.bass as bass
import concourse.mybir as mybir
import concourse.tile as tile
from concourse.bass_utils import run_bass_kernel_spmd

B, N, C, H, DH = 2, 2048, 1024, 8, 128
NCORES, G = 8, 4
CHUNK = N // G  # tokens per core
KT = C // 128  # 128-wide channel tiles
NJ = N // 128  # kv token tiles
SCALE = 1.0 / float(np.sqrt(DH))
F16, F32 = mybir.dt.float16, mybir.dt.float32
AF = mybir.ActivationFunctionType


def _split_excess_waits(nc):
    """This container's walrus caps sync-waits at 1 per plain instruction
    (2 for EventSemaphore) but Tile's scheduler attaches as many as an
    instruction needs. Hoist excess semaphore waits onto NoOps inserted
    just before the instruction on the same engine queue."""
    seq = [0]
    for f in nc.m.functions:
        for bb in f.blocks:
            out = []
            for ins in bb.instructions:
                si = ins.sync_info
                if si is None:
                    out.append(ins)
                    continue
                waits = list(si.on_wait)
                cap = 2 if isinstance(ins, mybir.InstEventSemaphore) else 1
                if len(waits) > cap and ins.engine != mybir.EngineType.Unassigned:
                    movable = [w for w in waits if w.sync_type == "semaphore"]
                    keep = [w for w in waits if w.sync_type != "semaphore"]
                    nkeep = cap - len(keep)
                    assert nkeep >= 0, f"{ins.name}: non-sem waits exceed cap"
                    if nkeep > 0:
                        keep += movable[-nkeep:]
                        movable = movable[:-nkeep]
                    for w in movable:
                        seq[0] += 1
                        nop = mybir.InstNoOp(
                            name=f"wsplit_{seq[0]}_{ins.name}", ins=[], outs=[])
                        nop.engine = ins.engine
                        nop.sync_info = bass_rust.SyncInfo(
                            on_wait=[w], on_update=[])
                        out.append(nop)
                    ins.sync_info = bass_rust.SyncInfo(
                        on_wait=keep, on_update=list(si.on_update))
                out.append(ins)
            bb.instructions = out


def build_nc(reps: int = 1):
    nc = bass.Bass("TRN2", target_bir_lowering=False, debug=False, num_devices=NCORES)

    ap = {}
    for name, shape, dt in [
        ("xqT", [C, CHUNK], F16),
        ("xkvT", [C, CHUNK], F16),
        ("wq", [C, C], F16),
        ("wk", [C, C], F16),
        ("wv", [C, C], F16),
        ("wo", [C, C], F16),
        ("bq_col", [128, KT], F32),
        ("bk_col", [128, KT], F32),
        ("bv_row", [128, C], F32),
        ("bo_row", [128, C], F32),
        ("ones_col", [128, 1], F16),
        ("ones_row", [1, 128], F16),
    ]:
        ap[name] = nc.dram_tensor(name, shape, dt, kind="ExternalInput").ap()
    out_ap = nc.dram_tensor("out", [CHUNK, C], F32, kind="ExternalOutput").ap()

    with tile.TileContext(nc) as tc:
        with (
            tc.tile_pool(name="const", bufs=1) as pconst,
            tc.tile_pool(name="w", bufs=1) as pw,
            tc.tile_pool(name="xT", bufs=1) as pxT,
            tc.tile_pool(name="acts", bufs=1) as pact,
            tc.tile_pool(name="kvh", bufs=2) as pkvh,
            tc.tile_pool(name="E", bufs=2) as pE,
            tc.tile_pool(name="small", bufs=2) as psmall,
            tc.tile_pool(name="outp", bufs=3) as pout,
            tc.tile_pool(name="psA", bufs=2, space="PSUM") as psA,
            tc.tile_pool(name="psS", bufs=2, space="PSUM") as psS,
            tc.tile_pool(name="psC", bufs=2, space="PSUM") as psC,
            tc.tile_pool(name="dram", bufs=1, space="DRAM") as pdram,
        ):

            def body():
                _emit(nc, ap, out_ap, pconst, pw, pxT, pact, pkvh, pE, psmall,
                      pout, psA, psS, psC, pdram)

            for _ in range(reps):
                body()
    _split_excess_waits(nc)
    return nc


def _emit(nc, ap, out_ap, pconst, pw, pxT, pact, pkvh, pE, psmall, pout,
          psA, psS, psC, pdram):
    dma = nc.gpsimd.dma_start

    ones_c = pconst.tile([128, 1], F16, tag="ones_c", name="ones_c")
    dma(ones_c[:], ap["ones_col"])
    ones_r = pconst.tile([1, 128], F16, tag="ones_r", name="ones_r")
    dma(ones_r[:], ap["ones_row"])
    bq_sb = pconst.tile([128, KT], F32, tag="bq_sb", name="bq_sb")
    dma(bq_sb[:], ap["bq_col"])
    bk_sb = pconst.tile([128, KT], F32, tag="bk_sb", name="bk_sb")
    dma(bk_sb[:], ap["bk_col"])
    bv_sb = pconst.tile([128, C], F32, tag="bv_sb", name="bv_sb")
    dma(bv_sb[:], ap["bv_row"])
    bo_sb = pconst.tile([128, C], F32, tag="bo_sb", name="bo_sb")
    dma(bo_sb[:], ap["bo_row"])

    # Preload the exp ACT table while input DMAs run.
    dummy = psmall.tile([1, 8], F32, tag="dummy", name="dummy")
    nc.scalar.activation(dummy[:], ones_r[:, 0:8], AF.Exp)

    # x^T chunks, laid out [128, (k, tok)]: column block k holds channel
    # rows k*128..(k+1)*128 of x^T.
    xkvT_sb = pxT.tile([128, KT * CHUNK], F16, tag="xkvT", name="xkvT")
    dma(xkvT_sb[:].rearrange("p (k t) -> p k t", k=KT),
        ap["xkvT"].rearrange("(k p) t -> p k t", p=128))
    xqT_sb = pxT.tile([128, KT * CHUNK], F16, tag="xqT", name="xqT")
    dma(xqT_sb[:].rearrange("p (k t) -> p k t", k=KT),
        ap["xqT"].rearrange("(k p) t -> p k t", p=128))

    def load_w(name):
        ts = []
        for k in range(KT):
            t = pw.tile([128, C], F16, tag=f"{name}{k}", name=f"{name}{k}")
            dma(t[:], ap[name][k * 128:(k + 1) * 128, :])
            ts.append(t)
        return ts

    wk_sb = load_w("wk")
    wv_sb = load_w("wv")
    wq_sb = load_w("wq")
    wo_sb = load_w("wo")

    kT_loc = pdram.tile([C, CHUNK], F16, tag="kT_loc", name="kT_loc")
    kT_g = pdram.tile([G * C, CHUNK], F16, tag="kT_g", name="kT_g")
    v_loc = pdram.tile([CHUNK, C], F16, tag="v_loc", name="v_loc")
    v_g = pdram.tile([G * CHUNK, C], F16, tag="v_g", name="v_g")

    # K^T projection: kT[m-block, tok] = sum_k Wk[k,m]^T x^T[k, tok] (+bk)
    kT_all = pact.tile([128, KT * CHUNK], F16, tag="kT_all", name="kT_all")
    for m in range(KT):
        ps = psA.tile([128, 512], F32, tag="ps", name="ps")
        for k in range(KT):
            nc.tensor.matmul(ps[:], wk_sb[k][:, m * 128:(m + 1) * 128],
                             xkvT_sb[:, k * CHUNK:(k + 1) * CHUNK],
                             start=(k == 0), stop=(k == KT - 1))
        nc.scalar.activation(kT_all[:, m * CHUNK:(m + 1) * CHUNK], ps[:],
                             AF.Identity, bias=bk_sb[:, m:m + 1])
        dma(kT_loc[m * 128:(m + 1) * 128, :], kT_all[:, m * CHUNK:(m + 1) * CHUNK])

    # V projection, token-major: v[tok, ch] = sum_k x^T[k, tok]^T Wv[k, ch]
    v_all = [pact.tile([128, C], F16, tag=f"v_all{mt}", name=f"v_all{mt}") for mt in range(4)]
    for mt in range(4):
        for n in range(2):
            ps = psA.tile([128, 512], F32, tag="ps", name="ps")
            for k in range(KT):
                nc.tensor.matmul(
                    ps[:],
                    xkvT_sb[:, k * CHUNK + mt * 128:k * CHUNK + (mt + 1) * 128],
                    wv_sb[k][:, n * 512:(n + 1) * 512],
                    start=(k == 0), stop=(k == KT - 1))
            nc.vector.tensor_add(v_all[mt][:, n * 512:(n + 1) * 512], ps[:],
                                 bv_sb[:, n * 512:(n + 1) * 512])
            dma(v_loc[mt * 128:(mt + 1) * 128, n * 512:(n + 1) * 512],
                v_all[mt][:, n * 512:(n + 1) * 512])

    rg = [[0, 1, 2, 3], [4, 5, 6, 7]]
    nc.gpsimd.collective_compute("AllGather", mybir.AluOpType.bypass,
                                 replica_groups=rg, ins=[kT_loc.opt()],
                                 outs=[kT_g.opt()])
    nc.gpsimd.collective_compute("AllGather", mybir.AluOpType.bypass,
                                 replica_groups=rg, ins=[v_loc.opt()],
                                 outs=[v_g.opt()])

    # Q^T projection (overlaps the AllGather on the PE).
    qT_all = pact.tile([128, KT * CHUNK], F16, tag="qT_all", name="qT_all")
    for m in range(KT):
        ps = psA.tile([128, 512], F32, tag="ps", name="ps")
        for k in range(KT):
            nc.tensor.matmul(ps[:], wq_sb[k][:, m * 128:(m + 1) * 128],
                             xqT_sb[:, k * CHUNK:(k + 1) * CHUNK],
                             start=(k == 0), stop=(k == KT - 1))
        nc.scalar.activation(qT_all[:, m * CHUNK:(m + 1) * CHUNK], ps[:],
                             AF.Identity, bias=bq_sb[:, m:m + 1])

    ctxT_all = pact.tile([128, H * CHUNK], F16, tag="ctxT_all", name="ctxT_all")
    for h in range(H):
        kTh = pkvh.tile([128, N], F16, tag="kTh", name="kTh")
        for g in range(G):
            dma(kTh[:, g * CHUNK:(g + 1) * CHUNK],
                kT_g[g * C + h * 128:g * C + (h + 1) * 128, :])
        vh = pkvh.tile([128, N], F16, tag="vh", name="vh")
        dma(vh[:].rearrange("p (j c) -> p j c", j=NJ),
            v_g.rearrange("(j p) c -> p j c", p=128)[:, :, h * DH:(h + 1) * DH])

        qTh = qT_all[:, h * CHUNK:(h + 1) * CHUNK]
        E = pE.tile([128, NJ * CHUNK], F16, tag="E", name="E")
        for jj in range(NJ // 2):
            Sp = psS.tile([128, 1024], F32, tag="S", name="S")
            for u in range(2):
                j = jj * 2 + u
                nc.tensor.matmul(Sp[:, u * 512:(u + 1) * 512],
                                 kTh[:, j * 128:(j + 1) * 128], qTh,
                                 start=True, stop=True)
            nc.scalar.activation(E[:, jj * 1024:(jj + 1) * 1024], Sp[:],
                                 AF.Exp, scale=SCALE)

        Esum = psmall.tile([128, CHUNK], F16, tag="Esum", name="Esum")
        nc.vector.tensor_add(Esum[:], E[:, 0:CHUNK], E[:, CHUNK:2 * CHUNK])
        for j in range(2, NJ):
            nc.vector.tensor_add(Esum[:], Esum[:], E[:, j * CHUNK:(j + 1) * CHUNK])

        ctxp = psC.tile([128, CHUNK], F32, tag="ctx", name="ctx")
        for j in range(NJ):
            nc.tensor.matmul(ctxp[:], vh[:, j * 128:(j + 1) * 128],
                             E[:, j * CHUNK:(j + 1) * CHUNK],
                             start=(j == 0), stop=(j == NJ - 1))

        denp = psA.tile([128, 512], F32, tag="ps", name="ps")
        nc.tensor.matmul(denp[0:1, :], ones_c[:], Esum[:], start=True, stop=True)
        recip = psmall.tile([1, CHUNK], F16, tag="recip", name="recip")
        with nc.allow_low_precision("softmax denom recip in f16; tol 2e-2"):
            nc.vector.reciprocal(recip[:], denp[0:1, :])
        bcastp = psA.tile([128, 512], F32, tag="ps", name="ps")
        nc.tensor.matmul(bcastp[:], ones_r[:], recip[:], start=True, stop=True)
        bcast_sb = psmall.tile([128, CHUNK], F16, tag="bcast", name="bcast")
        nc.scalar.copy(bcast_sb[:], bcastp[:])
        nc.vector.tensor_mul(ctxT_all[:, h * CHUNK:(h + 1) * CHUNK], ctxp[:],
                             bcast_sb[:])

    # Output projection: out[tok, ch] = sum_h ctx^T[h, tok]^T Wo[h, ch] (+bo)
    for mt in range(4):
        for n in range(2):
            po = psA.tile([128, 512], F32, tag="ps", name="ps")
            for k in range(KT):
                nc.tensor.matmul(
                    po[:],
                    ctxT_all[:, k * CHUNK + mt * 128:k * CHUNK + (mt + 1) * 128],
                    wo_sb[k][:, n * 512:(n + 1) * 512],
                    start=(k == 0), stop=(k == KT - 1))
            osb = pout.tile([128, 512], F32, tag="osb", name="osb")
            nc.vector.tensor_add(osb[:], po[:], bo_sb[:, n * 512:(n + 1) * 512])
            dma(out_ap[mt * 128:(mt + 1) * 128, n * 512:(n + 1) * 512], osb[:])


def prep_in_maps(inputs_q, inputs_kv, Wq, bq, Wk, bk, Wv, bv, Wo, bo):
    """Host-side layout prep: per-core chunk slicing, transpose to
    feature-major, fp16 casts, bias layout tiles. No FLOPs beyond casts."""
    inputs_q = np.asarray(inputs_q, dtype=np.float32)
    inputs_kv = np.asarray(inputs_kv, dtype=np.float32)
    w16 = {
        "wq": np.ascontiguousarray(np.asarray(Wq, np.float32).astype(np.float16)),
        "wk": np.ascontiguousarray(np.asarray(Wk, np.float32).astype(np.float16)),
        "wv": np.ascontiguousarray(np.asarray(Wv, np.float32).astype(np.float16)),
        "wo": np.ascontiguousarray(np.asarray(Wo, np.float32).astype(np.float16)),
    }
    bq = np.asarray(bq, np.float32)
    bk = np.asarray(bk, np.float32)
    bv = np.asarray(bv, np.float32)
    bo = np.asarray(bo, np.float32)
    shared = {
        **w16,
        "bq_col": np.ascontiguousarray(bq.reshape(KT, 128).T),
        "bk_col": np.ascontiguousarray(bk.reshape(KT, 128).T),
        "bv_row": np.ascontiguousarray(np.broadcast_to(bv, (128, C))),
        "bo_row": np.ascontiguousarray(np.broadcast_to(bo, (128, C))),
        "ones_col": np.ones((128, 1), np.float16),
        "ones_row": np.ones((1, 128), np.float16),
    }
    in_maps = []
    for c in range(NCORES):
        b, r = divmod(c, G)
        sl = slice(r * CHUNK, (r + 1) * CHUNK)
        in_maps.append({
            "xqT": np.ascontiguousarray(inputs_q[b, sl].T.astype(np.float16)),
            "xkvT": np.ascontiguousarray(inputs_kv[b, sl].T.astype(np.float16)),
            **shared,
        })
    return in_maps


def kernel(inputs_q, inputs_kv, Wq, bq, Wk, bk, Wv, bv, Wo, bo):
    in_maps = prep_in_maps(inputs_q, inputs_kv, Wq, bq, Wk, bk, Wv, bv, Wo, bo)
    nc = build_nc(reps=1)
    res = run_bass_kernel_spmd(nc, in_maps, core_ids=list(range(NCORES)))
    out = np.empty((B, N, C), np.float32)
    for c in range(NCORES):
        b, r = divmod(c, G)
        out[b, r * CHUNK:(r + 1) * CHUNK] = res.results[c]["out"]
    return out


if __name__ == "__main__":
    rng = np.random.default_rng(0)
    s = 1.0 / np.sqrt(C)
    ins = {
        "inputs_q": rng.standard_normal((B, N, C), np.float32),
        "inputs_kv": rng.standard_normal((B, N, C), np.float32),
        "Wq": rng.standard_normal((C, C), np.float32) * s,
        "bq": np.zeros(C, np.float32),
        "Wk": rng.standard_normal((C, C), np.float32) * s,
        "bk": np.zeros(C, np.float32),
        "Wv": rng.standard_normal((C, C), np.float32) * s,
        "bv": np.zeros(C, np.float32),
        "Wo": rng.standard_normal((C, C), np.float32) * s,
        "bo": np.zeros(C, np.float32),
    }
    out = kernel(**ins)
    print("out", out.shape, out.dtype, np.abs(out).mean())
